# revision 2
# baseline (speedup 1.0000x reference)
"""Trainium2 Bass kernel for continuous-filter convolution (gnn message passing).

Reference computation (shapes hardcoded):
    features [2,256,32] f32, geometry [2,256,3] f32, centers [16] f32,
    kernel_w [16,32,32] f32, n_norm scalar
    d[z,a,b]   = sqrt(sum_c (g[z,b,c]-g[z,a,c])^2 + 1e-9)
    rbf        = exp(-10*(d[...,None]-centers)^2)            [z,a,b,n]
    k          = einsum('zabn,nij->zabij', rbf, kernel_w) / sqrt(n_norm)
    out[z,a,i] = einsum('zabij,zbj->zai', k, features)

Key restructuring: k is never materialized. Instead
    m[z,b,n,i]  = sum_j kernel_w[n,i,j] * features[z,b,j] / sqrt(n_norm)
    out[z,a,i]  = sum_{b,n} rbf[z,a,b,n] * m[z,b,n,i]
which is a [64 x 4096] @ [4096 x 32] contraction per (z, a-slice).

Sharding: 8 cores = 2 batches x 4 a-slices of 64 points. Each core gets its
geometry slice plus replicated features/weights; no cross-device reduction.
"""

import numpy as np
from contextlib import ExitStack

import concourse.bass as bass
import concourse.tile as tile
from concourse import mybir
from concourse.bass_utils import run_bass_kernel_spmd

GAMMA = 10.0
EPS = 1e-9
B, P, C = 2, 256, 32
NB, I, J = 16, 32, 32
NCORES = 8
AS = NCORES // B  # a-slices per batch = 4
AL = P // AS      # points per a-slice = 64

f32 = mybir.dt.float32


def _split_multi_waits(nc):
    """This walrus build only lowers one sync wait per instruction; Tile's
    scheduler attaches several to the tail drain. Hoist extras into
    single-wait EventSemaphore instructions on the same engine."""
    n = 0
    for fn in nc.m.functions:
        for bb in fn.blocks:
            insts = list(bb.instructions)
            new = []
            for inst in insts:
                si = getattr(inst, "sync_info", None)
                if si is not None and si.on_wait and len(si.on_wait) > 1:
                    waits = list(si.on_wait)
                    for w in waits[:-1]:
                        n += 1
                        new.append(
                            mybir.InstEventSemaphore(
                                name=f"I-msplit{n}",
                                engine=inst.engine,
                                sync_info=mybir.SyncInfo(on_wait=[w], on_update=[]),
                            )
                        )
                    inst.sync_info = mybir.SyncInfo(
                        on_wait=[waits[-1]], on_update=list(si.on_update or [])
                    )
                new.append(inst)
            try:
                bb.instructions = new
            except Exception:
                bb.instructions.clear()
                for i in new:
                    bb.add_instruction(i)
    return n


def _build_program():
    nc = bass.Bass()
    g_gb = nc.declare_dram_parameter("gb", [P, 3], f32, isOutput=False)
    g_gab = nc.declare_dram_parameter("gab", [128, AL * 3], f32, isOutput=False)
    g_ft = nc.declare_dram_parameter("ft", [J, P], f32, isOutput=False)
    g_wt = nc.declare_dram_parameter("wt", [J, NB * I], f32, isOutput=False)
    g_cb = nc.declare_dram_parameter("cb", [128, NB], f32, isOutput=False)
    g_out = nc.declare_dram_parameter("out", [AL, I], f32, isOutput=True)

    Act = mybir.ActivationFunctionType
    Alu = mybir.AluOpType

    # activation() requires float biases as pre-registered const APs
    eps_t = nc.alloc_sbuf_tensor(f"const-float32-{EPS}", [128, 1], f32)
    nc.gpsimd.memset(eps_t.ap(), EPS)
    nc.const_aps.aps[(f32, EPS)] = eps_t.ap()
    nc.all_engine_barrier()

    with ExitStack() as ctx:
        tc = ctx.enter_context(tile.TileContext(nc))
        pool = ctx.enter_context(tc.tile_pool(name="sb", bufs=1))
        ppool = ctx.enter_context(tc.tile_pool(name="ps", bufs=1, space="PSUM"))

        t_ft = pool.tile([J, P], f32, tag="ft")
        nc.sync.dma_start(t_ft[:], g_ft[:])
        t_wt = pool.tile([J, NB * I], f32, tag="wt")
        nc.sync.dma_start(t_wt[:], g_wt[:])
        t_cb = pool.tile([128, NB], f32, tag="cb")
        nc.sync.dma_start(t_cb[:], g_cb[:])
        t_gab = pool.tile([128, AL, 3], f32, tag="gab")
        nc.sync.dma_start(t_gab[:], g_gab[:].rearrange("p (a c) -> p a c", c=3))
        t_gb = []
        for ch in range(2):
            t = pool.tile([128, 3], f32, tag=f"gb{ch}")
            nc.sync.dma_start(t[:], g_gb[ch * 128 : (ch + 1) * 128, :])
            t_gb.append(t)

        # m[b, (n,i)] = sum_j ft[j,b] * wt[j,(n,i)]  (features pre-scaled by
        # 1/sqrt(n_norm) on host)
        t_m = []
        for ch in range(2):
            pm = ppool.tile([128, NB * I], f32, tag=f"pm{ch}")
            nc.tensor.matmul(
                pm[:],
                lhsT=t_ft[:, ch * 128 : (ch + 1) * 128],
                rhs=t_wt[:],
                start=True,
                stop=True,
            )
            ms = pool.tile([128, NB * I], f32, tag=f"m{ch}")
            nc.vector.tensor_copy(ms[:], pm[:])
            t_m.append(ms)

        po = ppool.tile([AL, I], f32, tag="po")
        for ch in range(2):
            # diff[b, a, c] = ga[a,c] - gb[b,c]; direct form keeps the a==b
            # diagonal exactly zero (expansion form would lose it to
            # cancellation and amplify through sqrt at d~0).
            diff = pool.tile([128, AL, 3], f32, tag="diff")
            nc.vector.tensor_sub(
                diff[:],
                t_gab[:],
                t_gb[ch][:].unsqueeze(1).broadcast_to([128, AL, 3]),
            )
            sq = pool.tile([128, AL, 3], f32, tag="sqd")
            nc.vector.tensor_mul(sq[:], diff[:], diff[:])
            d2 = pool.tile([128, AL], f32, tag="d2")
            nc.vector.tensor_reduce(d2[:], sq[:], axis=mybir.AxisListType.X, op=Alu.add)
            d = pool.tile([128, AL], f32, tag="d")
            nc.scalar.activation(d[:], d2[:], Act.Sqrt, bias=EPS)
            # tt[b, n, a] = d[b,a] - centers[n]
            tt = pool.tile([128, NB, AL], f32, tag="tt")
            nc.vector.tensor_sub(
                tt[:],
                d[:].unsqueeze(1).broadcast_to([128, NB, AL]),
                t_cb[:].unsqueeze(2).broadcast_to([128, NB, AL]),
            )
            sq2 = pool.tile([128, NB, AL], f32, tag="sq2")
            nc.scalar.activation(sq2[:], tt[:], Act.Square)
            rbf = pool.tile([128, NB, AL], f32, tag="rbf")
            nc.scalar.activation(rbf[:], sq2[:], Act.Exp, scale=-GAMMA)
            # out[a,i] += sum_b rbf[b, n, a] * m[b, (n,i)] for each n
            for n in range(NB):
                nc.tensor.matmul(
                    po[:],
                    lhsT=rbf[:, n, :],
                    rhs=t_m[ch][:, n * I : (n + 1) * I],
                    start=(ch == 0 and n == 0),
                    stop=(ch == 1 and n == NB - 1),
                )
        t_o = pool.tile([AL, I], f32, tag="o")
        nc.vector.tensor_copy(t_o[:], po[:])
        nc.sync.dma_start(g_out[:], t_o[:])

    _split_multi_waits(nc)
    return nc


_NC = None


def kernel(features, geometry, centers, kernel_w, n_norm):
    global _NC
    if _NC is None:
        _NC = _build_program()

    features = np.asarray(features, np.float32)
    geometry = np.asarray(geometry, np.float32)
    centers = np.asarray(centers, np.float32)
    kernel_w = np.asarray(kernel_w, np.float32)
    scale = 1.0 / np.sqrt(float(np.asarray(n_norm).item()))

    # wt[j, n*I+i] = kernel_w[n,i,j]
    wt = np.ascontiguousarray(kernel_w.transpose(2, 0, 1).reshape(J, NB * I))
    cb = np.ascontiguousarray(np.broadcast_to(centers.reshape(1, NB), (128, NB)))

    in_maps = []
    for core in range(NCORES):
        z, sl = divmod(core, AS)
        ga = geometry[z, sl * AL : (sl + 1) * AL, :]  # [AL, 3]
        gab = np.ascontiguousarray(
            np.broadcast_to(ga.reshape(1, AL * 3), (128, AL * 3))
        )
        ft = np.ascontiguousarray(features[z].T) * scale  # [J, P]
        in_maps.append(
            {
                "gb": np.ascontiguousarray(geometry[z]),
                "gab": gab,
                "ft": ft.astype(np.float32),
                "wt": wt,
                "cb": cb,
            }
        )

    res = run_bass_kernel_spmd(_NC, in_maps, list(range(NCORES)))

    out = np.empty((B, P, I), np.float32)
    for core in range(NCORES):
        z, sl = divmod(core, AS)
        out[z, sl * AL : (sl + 1) * AL, :] = res.results[core]["out"]
    return out


# revision 5
# speedup vs baseline: 1.1812x; 1.1812x over previous
"""Trainium2 Bass kernel for continuous-filter convolution (gnn message passing).

Reference computation (shapes hardcoded):
    features [2,256,32] f32, geometry [2,256,3] f32, centers [16] f32,
    kernel_w [16,32,32] f32, n_norm scalar
    d[z,a,b]   = sqrt(sum_c (g[z,b,c]-g[z,a,c])^2 + 1e-9)
    rbf        = exp(-10*(d[...,None]-centers)^2)            [z,a,b,n]
    k          = einsum('zabn,nij->zabij', rbf, kernel_w) / sqrt(n_norm)
    out[z,a,i] = einsum('zabij,zbj->zai', k, features)

Key restructuring: k is never materialized. Instead
    m[z,b,n,i]  = sum_j kernel_w[n,i,j] * features[z,b,j] / sqrt(n_norm)
    out[z,a,i]  = sum_{b,n} rbf[z,a,b,n] * m[z,b,n,i]
which is a [64 x 4096] @ [4096 x 32] contraction per (z, a-slice).

Sharding: 8 cores = 2 batches x 4 a-slices of 64 points. Each core gets its
geometry slice plus replicated features/weights; no cross-device reduction.

Device layout per core (b = point index, chunked 2x128 over partitions):
    d[b, (ch,a)]    distance from point b to the core's 64 a-points
    rbf[b, (n,a)]   radial basis values, computed per (ch, n-half) unit and
                    pipelined DVE(sub) -> GpSimd(square) -> ACT(exp) -> PE
    out[a,i]        accumulated over 32 PE matmuls (K=128 each)
"""

import numpy as np
from contextlib import ExitStack

import concourse.bass as bass
import concourse.tile as tile
from concourse import mybir
from concourse.bass_utils import run_bass_kernel_spmd

GAMMA = 10.0
EPS = 1e-9
B, P, C = 2, 256, 32
NB, I, J = 16, 32, 32
NCORES = 8
AS = NCORES // B  # a-slices per batch = 4
AL = P // AS      # points per a-slice = 64
NH = NB // 2      # n-half size = 8

f32 = mybir.dt.float32

# packed input A: [128, 215] = gab(192) | gb6(6) | cb(16) | eps(1)
GAB0, GB0, CB0, EPS0 = 0, 192, 198, 214
WA = 215
# packed input B: [32, 768] = ft(256) | wt(512)
FT0, WT0 = 0, 256
WB = 768


def _split_multi_waits(nc):
    """This walrus build only lowers one sync wait per instruction; Tile's
    scheduler attaches several to some instructions (notably the tail drain).
    Hoist extras into single-wait EventSemaphore instructions just before, on
    the same engine — semantically identical, sequencer waits then issues."""
    n = 0
    for fn in nc.m.functions:
        for bb in fn.blocks:
            insts = list(bb.instructions)
            new = []
            for inst in insts:
                si = getattr(inst, "sync_info", None)
                if si is not None and si.on_wait and len(si.on_wait) > 1:
                    waits = list(si.on_wait)
                    for w in waits[:-1]:
                        n += 1
                        new.append(
                            mybir.InstEventSemaphore(
                                name=f"I-msplit{n}",
                                engine=inst.engine,
                                sync_info=mybir.SyncInfo(on_wait=[w], on_update=[]),
                            )
                        )
                    inst.sync_info = mybir.SyncInfo(
                        on_wait=[waits[-1]], on_update=list(si.on_update or [])
                    )
                new.append(inst)
            try:
                bb.instructions = new
            except Exception:
                bb.instructions.clear()
                for i in new:
                    bb.add_instruction(i)
    return n


def _build_program():
    nc = bass.Bass()
    g_a = nc.declare_dram_parameter("ina", [128, WA], f32, isOutput=False)
    g_b = nc.declare_dram_parameter("inb", [J, WB], f32, isOutput=False)
    g_out = nc.declare_dram_parameter("out", [AL, I], f32, isOutput=True)

    Act = mybir.ActivationFunctionType
    Alu = mybir.AluOpType

    with ExitStack() as ctx:
        tc = ctx.enter_context(tile.TileContext(nc))
        pool = ctx.enter_context(tc.tile_pool(name="sb", bufs=1))
        pipe = ctx.enter_context(tc.tile_pool(name="pipe", bufs=2))
        ppool = ctx.enter_context(tc.tile_pool(name="ps", bufs=1, space="PSUM"))

        t_a = pool.tile([128, WA], f32, tag="ina")
        nc.sync.dma_start(t_a[:], g_a[:])
        t_b = pool.tile([J, WB], f32, tag="inb")
        nc.gpsimd.dma_start(t_b[:], g_b[:])

        gab = t_a[:, GAB0 : GAB0 + AL * 3].rearrange("p (a c) -> p a c", c=3)
        gb6 = t_a[:, GB0 : GB0 + 6].rearrange("p (h c) -> p h c", c=3)
        cb = t_a[:, CB0 : CB0 + NB]
        epsc = t_a[:, EPS0 : EPS0 + 1]

        # distance chain, both chunks fused: d[b, (ch, a)]
        diff = pool.tile([128, 2 * AL * 3], f32, tag="diff")
        diff3 = diff[:].rearrange("p (h a c) -> p h a c", h=2, c=3)
        nc.vector.tensor_sub(
            diff3,
            gab.unsqueeze(1).broadcast_to([128, 2, AL, 3]),
            gb6.unsqueeze(2).broadcast_to([128, 2, AL, 3]),
        )
        sqd = pool.tile([128, 2 * AL * 3], f32, tag="sqd")
        sqd3 = sqd[:].rearrange("p (h a c) -> p h a c", h=2, c=3)
        nc.vector.tensor_mul(sqd3, diff3, diff3)
        d2 = pool.tile([128, 2 * AL], f32, tag="d2")
        nc.vector.tensor_reduce(
            d2[:].rearrange("p (h a) -> p h a", h=2), sqd3, axis=mybir.AxisListType.X,
            op=Alu.add,
        )
        d = pool.tile([128, 2 * AL], f32, tag="d")
        nc.scalar.activation(d[:], d2[:], Act.Sqrt, bias=epsc)

        # m[b, (n,i)] per chunk; PSUM -> SBUF via DMA to keep ACT/DVE free
        t_m = []
        for ch in range(2):
            pm = ppool.tile([128, NB * I], f32, tag=f"pm{ch}")
            nc.tensor.matmul(
                pm[:],
                lhsT=t_b[:, FT0 + ch * 128 : FT0 + (ch + 1) * 128],
                rhs=t_b[:, WT0 : WT0 + NB * I],
                start=True,
                stop=True,
            )
            ms = pool.tile([128, NB * I], f32, tag=f"m{ch}")
            if ch == 0:
                nc.scalar.copy(ms[:], pm[:])
            else:
                nc.vector.tensor_copy(ms[:], pm[:])
            t_m.append(ms)

        # rbf + contraction, pipelined in 4 units of (chunk, n-half)
        po = ppool.tile([AL, I], f32, tag="po")
        first = True
        for ch in range(2):
            for h in range(2):
                tt = pipe.tile([128, NH * AL], f32, tag="tt")
                nc.vector.tensor_sub(
                    tt[:].rearrange("p (n a) -> p n a", n=NH),
                    d[:, ch * AL : (ch + 1) * AL]
                    .unsqueeze(1)
                    .broadcast_to([128, NH, AL]),
                    cb[:, h * NH : (h + 1) * NH]
                    .unsqueeze(2)
                    .broadcast_to([128, NH, AL]),
                )
                sq2 = pipe.tile([128, NH * AL], f32, tag="sq2")
                nc.gpsimd.tensor_mul(sq2[:], tt[:], tt[:])
                rbf = pipe.tile([128, NH * AL], f32, tag="rbf")
                nc.scalar.activation(rbf[:], sq2[:], Act.Exp, scale=-GAMMA)
                for k in range(NH):
                    n = h * NH + k
                    nc.tensor.matmul(
                        po[:],
                        lhsT=rbf[:, k * AL : (k + 1) * AL],
                        rhs=t_m[ch][:, n * I : (n + 1) * I],
                        start=first,
                        stop=(ch == 1 and n == NB - 1),
                    )
                    first = False
        t_o = pool.tile([AL, I], f32, tag="o")
        nc.vector.tensor_copy(t_o[:], po[:])
        nc.sync.dma_start(g_out[:], t_o[:])

    _split_multi_waits(nc)
    return nc


_NC = None


def _pack_inputs(features, geometry, centers, kernel_w, n_norm):
    features = np.asarray(features, np.float32)
    geometry = np.asarray(geometry, np.float32)
    centers = np.asarray(centers, np.float32)
    kernel_w = np.asarray(kernel_w, np.float32)
    scale = 1.0 / np.sqrt(float(np.asarray(n_norm).item()))

    wt = np.ascontiguousarray(kernel_w.transpose(2, 0, 1).reshape(J, NB * I))
    in_maps = []
    for core in range(NCORES):
        z, sl = divmod(core, AS)
        ina = np.empty((128, WA), np.float32)
        ga = geometry[z, sl * AL : (sl + 1) * AL, :]  # [AL, 3]
        ina[:, GAB0 : GAB0 + AL * 3] = ga.reshape(1, AL * 3)
        ina[:, GB0 : GB0 + 6] = geometry[z].reshape(2, 128, 3).transpose(1, 0, 2).reshape(128, 6)
        ina[:, CB0 : CB0 + NB] = centers.reshape(1, NB)
        ina[:, EPS0] = EPS
        inb = np.empty((J, WB), np.float32)
        inb[:, FT0 : FT0 + P] = features[z].T * scale
        inb[:, WT0 : WT0 + NB * I] = wt
        in_maps.append({"ina": ina, "inb": inb})
    return in_maps


def kernel(features, geometry, centers, kernel_w, n_norm):
    global _NC
    if _NC is None:
        _NC = _build_program()

    in_maps = _pack_inputs(features, geometry, centers, kernel_w, n_norm)
    res = run_bass_kernel_spmd(_NC, in_maps, list(range(NCORES)))

    out = np.empty((B, P, I), np.float32)
    for core in range(NCORES):
        z, sl = divmod(core, AS)
        out[z, sl * AL : (sl + 1) * AL, :] = res.results[core]["out"]
    return out
